# revision 12
# baseline (speedup 1.0000x reference)
"""Trainium2 Bass kernel for sparse-attention MultiHeadAttention.

Strategy (8 NeuronCores, batch-parallel):
  - Core b handles batch element b end-to-end (B == n_cores == 8).
  - Host does index/layout prep only: transposes for matmul layouts,
    per-core edge partitioning (edges belong to core batch[src]),
    scatter-index computation, and a query-axis permutation that places
    core b's graph nodes first so the edge-bias window is [0, n_cap).
  - Device per core:
      qhT/khT = (Wq q_b^T), (Wk k_b^T)  as (HID x N) "head-transposed"
      vh      = v_b Wv^T                as (N x HID) (key-partition rows)
      ew      = edge_attr_aug @ WeT_aug (per-edge per-head bias values)
      scatter ew into a DRAM table at rows dst*n_cap + i_local (one
      indirect DMA; duplicate (src,dst) pairs pre-summed on device via a
      selection-matrix matmul within their chunk)
      scoresT[j,i] = sum_d khT[d,j] qhT[d,i]   (PE, per head, j-tiled)
      scoresT += edge bias (table readback) + (-1e9)*maskT  (DVE)
      attnT = exp(scoresT)  (ACT, no max-subtraction: inputs are tiny)
      out_augT[(d|1), i] += vh_aug[j, (d|1)]^T attnT[j, i]  (PE; the
      appended ones-column yields the softmax row sums for free)
      outT = out_augT[0:32] / rowsum (broadcast via DRAM roundtrip)
      y = outT_all^T stacked -> final Wo projection -> (N x HID) out
  - Host inverse-permutes rows and stacks core outputs.
"""

import math

import numpy as np

B, N, HID, H, EF = 8, 1024, 256, 8, 16
D = HID // H  # 32
P = 128
NT = N // P  # 8 j-tiles / n-tiles
SCALE = float(D) ** -0.5
NEG = -1.0e9

_PROGRAM_CACHE: dict = {}


def _build_program(n_cap: int, nc_u: int, nc_d: int):
    import concourse.bacc as bacc
    import concourse.bass as bass
    import concourse.mybir as mybir
    import concourse.tile as tile

    f32 = mybir.dt.float32
    i32 = mybir.dt.int32
    AF = mybir.ActivationFunctionType
    ALU = mybir.AluOpType

    NC = nc_u + nc_d  # total 128-edge chunks
    NE = NC * P
    ZCOLS = 2048
    ZROWS_F = P * ZCOLS  # floats zeroed per DMA
    TROWS = N * n_cap + 1
    TROWS_PAD = ((TROWS * 8 + ZROWS_F - 1) // ZROWS_F) * ZROWS_F // 8
    NZ = TROWS_PAD * 8 // ZROWS_F
    assert n_cap <= 512

    nc = bacc.Bacc(
        "TRN2",
        target_bir_lowering=False,
        debug=False,
        enable_asserts=False,
    )
    dp = nc.declare_dram_parameter
    qT = dp("qT", [HID, N], f32, isOutput=False)
    kT = dp("kT", [HID, N], f32, isOutput=False)
    vT = dp("vT", [HID, N], f32, isOutput=False)
    maskF = dp("maskF", [N, N], f32, isOutput=False)
    WqT = dp("WqT", [HID, HID], f32, isOutput=False)
    WkT = dp("WkT", [HID, HID], f32, isOutput=False)
    WvT = dp("WvT", [HID, HID], f32, isOutput=False)
    WoT = dp("WoT", [HID, HID], f32, isOutput=False)
    bq = dp("bq", [HID, 1], f32, isOutput=False)
    bk = dp("bk", [HID, 1], f32, isOutput=False)
    bv = dp("bv", [1, HID], f32, isOutput=False)
    bo = dp("bo", [1, HID], f32, isOutput=False)
    WeTa = dp("WeTa", [EF + 1, H], f32, isOutput=False)
    attrT = dp("attrT", [EF + 1, NE], f32, isOutput=False)
    idxT = dp("idxT", [P, NC], i32, isOutput=False)
    identI = dp("identI", [P, P], f32, isOutput=False)
    out = dp("out", [N, HID], f32, isOutput=True)

    with tile.TileContext(nc) as tc:
        with (
            tc.tile_pool(name="dram", bufs=1, space="DRAM") as dram,
            tc.tile_pool(name="dram_rs", bufs=2, space="DRAM") as dram_rs,
            tc.tile_pool(name="const", bufs=1) as cons,
            tc.tile_pool(name="pers", bufs=1) as pers,
        ):
            # ---------------- constants / weights ----------------
            ident = cons.tile([P, P], f32)
            nc.sync.dma_start(out=ident[:], in_=identI[:, :])
            wq_sb = [cons.tile([P, HID], f32, tag=f"wq{t}", name=f"wq{t}") for t in range(2)]
            wk_sb = [cons.tile([P, HID], f32, tag=f"wk{t}", name=f"wk{t}") for t in range(2)]
            wv_sb = [cons.tile([P, HID], f32, tag=f"wv{t}", name=f"wv{t}") for t in range(2)]
            wo_sb = [cons.tile([P, HID], f32, tag=f"wo{t}", name=f"wo{t}") for t in range(2)]
            for t in range(2):
                nc.sync.dma_start(out=wq_sb[t][:], in_=WqT[t * P : (t + 1) * P, :])
                nc.sync.dma_start(out=wk_sb[t][:], in_=WkT[t * P : (t + 1) * P, :])
                nc.sync.dma_start(out=wv_sb[t][:], in_=WvT[t * P : (t + 1) * P, :])
                nc.sync.dma_start(out=wo_sb[t][:], in_=WoT[t * P : (t + 1) * P, :])
            we_sb = cons.tile([EF + 1, H], f32)
            nc.sync.dma_start(out=we_sb[:], in_=WeTa[:, :])
            bq_sb = [cons.tile([P, 1], f32, tag=f"bq{t}", name=f"bqs{t}") for t in range(2)]
            bk_sb = [cons.tile([P, 1], f32, tag=f"bk{t}", name=f"bks{t}") for t in range(2)]
            for t in range(2):
                nc.sync.dma_start(out=bq_sb[t][:], in_=bq[t * P : (t + 1) * P, :])
                nc.sync.dma_start(out=bk_sb[t][:], in_=bk[t * P : (t + 1) * P, :])
            bv_bc = cons.tile([P, HID], f32)
            nc.sync.dma_start(out=bv_bc[:], in_=bv[0:1, :].to_broadcast((P, HID)))
            bo_bc = cons.tile([P, HID], f32)
            nc.sync.dma_start(out=bo_bc[:], in_=bo[0:1, :].to_broadcast((P, HID)))

            # ---------------- edge-bias table ----------------
            table = dram.tile([TROWS_PAD, 8], f32)
            tab_flat = table[:].rearrange("a b -> (a b)")
            ztile = pers.tile([P, ZCOLS], f32)
            nc.vector.memset(ztile[:], 0.0)
            zv = tab_flat.rearrange("(n p f) -> n p f", p=P, f=ZCOLS)
            for i in range(NZ):
                nc.gpsimd.dma_start(out=zv[i], in_=ztile[:])

            idx_sb = pers.tile([P, NC], i32)
            nc.sync.dma_start(out=idx_sb[:], in_=idxT[:, :])
            ew_sb = pers.tile([P, 8 * NC], f32)

            with (
                tc.tile_pool(name="attr", bufs=3) as attrp,
                tc.tile_pool(name="ewp", bufs=2, space="PSUM") as ewp,
                tc.tile_pool(name="dupp", bufs=2, space="PSUM") as dupp,
                tc.tile_pool(name="dups", bufs=3) as dups,
            ):
                APC = 16  # chunks per attr piece
                n_pieces = (NC + APC - 1) // APC
                for pc in range(n_pieces):
                    c0, c1 = pc * APC, min((pc + 1) * APC, NC)
                    at = attrp.tile([EF + 1, APC * P], f32, tag="attr")
                    nc.gpsimd.dma_start(
                        out=at[:, : (c1 - c0) * P], in_=attrT[:, c0 * P : c1 * P]
                    )
                    eps = ewp.tile([P, (c1 - c0) * 8], f32, tag="ewps")
                    for c in range(c0, c1):
                        lc = c - c0
                        nc.tensor.matmul(
                            out=eps[:, lc * 8 : lc * 8 + 8],
                            lhsT=at[:, lc * P : (lc + 1) * P],
                            rhs=we_sb[:],
                            start=True,
                            stop=True,
                        )
                    if c1 <= nc_u:
                        # unique chunks: bulk copy PSUM -> ew_sb
                        nc.scalar.activation(
                            out=ew_sb[:, c0 * 8 : c1 * 8], in_=eps[:], func=AF.Copy
                        )
                    else:
                        for c in range(c0, c1):
                            lc = c - c0
                            if c < nc_u:
                                nc.scalar.activation(
                                    out=ew_sb[:, c * 8 : c * 8 + 8],
                                    in_=eps[:, lc * 8 : lc * 8 + 8],
                                    func=AF.Copy,
                                )
                                continue
                            # duplicate-group chunk: pre-sum rows sharing an
                            # index with a selection-matrix matmul
                            ewt = dups.tile([P, 8], f32, tag="ewt")
                            nc.scalar.activation(
                                out=ewt[:], in_=eps[:, lc * 8 : lc * 8 + 8], func=AF.Copy
                            )
                            idf = dups.tile([P, 1], f32, tag="idf")
                            nc.vector.tensor_copy(out=idf[:], in_=idx_sb[:, c : c + 1])
                            tp = dupp.tile([P, P], f32, tag="tp")
                            nc.tensor.transpose(
                                out=tp[:], in_=idf[:].to_broadcast((P, P)), identity=ident[:]
                            )
                            idft = dups.tile([P, P], f32, tag="idft")
                            nc.scalar.activation(out=idft[:], in_=tp[:], func=AF.Copy)
                            sel = dups.tile([P, P], f32, tag="sel")
                            nc.vector.tensor_tensor(
                                out=sel[:],
                                in0=idf[:].to_broadcast((P, P)),
                                in1=idft[:],
                                op=ALU.is_equal,
                            )
                            sp = dupp.tile([P, 8], f32, tag="sp")
                            nc.tensor.matmul(
                                out=sp[:], lhsT=sel[:], rhs=ewt[:], start=True, stop=True
                            )
                            nc.scalar.activation(
                                out=ew_sb[:, c * 8 : c * 8 + 8], in_=sp[:], func=AF.Copy
                            )

            # scatter per 128-edge chunk (HW indirect DMA requires a
            # single-column offset table; fused multi-chunk scatters
            # mis-execute on hardware)
            for c in range(NC):
                nc.gpsimd.indirect_dma_start(
                    out=table[:],
                    out_offset=bass.IndirectOffsetOnAxis(ap=idx_sb[:, c : c + 1], axis=0),
                    in_=ew_sb[:, c * 8 : (c + 1) * 8],
                    in_offset=None,
                )

            # ---------------- projections ----------------
            qhT = [pers.tile([64, N], f32, tag=f"qhT{t}", name=f"qhT{t}") for t in range(4)]
            khT = [pers.tile([64, N], f32, tag=f"khT{t}", name=f"khT{t}") for t in range(4)]
            vha = [pers.tile([P, 33 * H], f32, tag=f"vha{j}", name=f"vha{j}") for j in range(NT)]

            with (
                tc.tile_pool(name="xin", bufs=2) as xin,
                tc.tile_pool(name="projp", bufs=2, space="PSUM") as projp,
            ):
                q_sb = [xin.tile([P, N], f32, tag=f"q{t}", name=f"qsb{t}") for t in range(2)]
                k_sb = [xin.tile([P, N], f32, tag=f"k{t}", name=f"ksb{t}") for t in range(2)]
                v_sb = [xin.tile([P, N], f32, tag=f"v{t}", name=f"vsb{t}") for t in range(2)]
                for t in range(2):
                    nc.sync.dma_start(out=q_sb[t][:], in_=qT[t * P : (t + 1) * P, :])
                    nc.sync.dma_start(out=k_sb[t][:], in_=kT[t * P : (t + 1) * P, :])
                    nc.sync.dma_start(out=v_sb[t][:], in_=vT[t * P : (t + 1) * P, :])

                for mt in range(2):
                    for ih in range(2):
                        ps = projp.tile([P, 512], f32, tag="prq")
                        for kt in range(2):
                            nc.tensor.matmul(
                                out=ps[:],
                                lhsT=wq_sb[kt][:, mt * P : (mt + 1) * P],
                                rhs=q_sb[kt][:, ih * 512 : (ih + 1) * 512],
                                start=(kt == 0),
                                stop=(kt == 1),
                            )
                        for half in range(2):
                            nc.scalar.activation(
                                out=qhT[2 * mt + half][:, ih * 512 : (ih + 1) * 512],
                                in_=ps[half * 64 : (half + 1) * 64, :],
                                func=AF.Identity,
                                bias=bq_sb[mt][half * 64 : (half + 1) * 64, :],
                                scale=SCALE,
                            )
                        ps2 = projp.tile([P, 512], f32, tag="prk")
                        for kt in range(2):
                            nc.tensor.matmul(
                                out=ps2[:],
                                lhsT=wk_sb[kt][:, mt * P : (mt + 1) * P],
                                rhs=k_sb[kt][:, ih * 512 : (ih + 1) * 512],
                                start=(kt == 0),
                                stop=(kt == 1),
                            )
                        for half in range(2):
                            nc.scalar.activation(
                                out=khT[2 * mt + half][:, ih * 512 : (ih + 1) * 512],
                                in_=ps2[half * 64 : (half + 1) * 64, :],
                                func=AF.Identity,
                                bias=bk_sb[mt][half * 64 : (half + 1) * 64, :],
                                scale=1.0,
                            )

                for jt in range(NT):
                    ps = projp.tile([P, HID], f32, tag="prv")
                    for kt in range(2):
                        nc.tensor.matmul(
                            out=ps[:],
                            lhsT=v_sb[kt][:, jt * P : (jt + 1) * P],
                            rhs=wv_sb[kt][:],
                            start=(kt == 0),
                            stop=(kt == 1),
                        )
                    for h in range(H):
                        nc.vector.tensor_tensor(
                            out=vha[jt][:, 33 * h : 33 * h + 32],
                            in0=ps[:, 32 * h : 32 * h + 32],
                            in1=bv_bc[:, 32 * h : 32 * h + 32],
                            op=ALU.add,
                        )
                    nc.vector.memset(vha[jt][:, 32::33], 1.0)

            # ---------------- mask + table readback ----------------
            mk = [pers.tile([P, N], f32, tag=f"mk{j}", name=f"mk{j}") for j in range(NT)]
            for jt in range(NT):
                nc.sync.dma_start(out=mk[jt][:], in_=maskF[jt * P : (jt + 1) * P, :])
            tb = [pers.tile([P, n_cap * 8], f32, tag=f"tb{j}", name=f"tbr{j}") for j in range(NT)]
            for jt in range(NT):
                nc.gpsimd.dma_start(
                    out=tb[jt][:],
                    in_=tab_flat[
                        jt * P * n_cap * 8 : (jt + 1) * P * n_cap * 8
                    ].rearrange("(p f) -> p f", f=n_cap * 8),
                )

            # ---------------- attention ----------------
            oT = [pers.tile([P, N], f32, tag=f"oT{t}", name=f"oT{t}") for t in range(2)]
            with (
                tc.tile_pool(name="scp", bufs=2, space="PSUM") as scp,
                tc.tile_pool(name="oap", bufs=2, space="PSUM") as oap,
                tc.tile_pool(name="att", bufs=3) as att,
                tc.tile_pool(name="rsp", bufs=2) as rsp,
            ):
                for h in range(H):
                    ht, hr = h // 2, (h % 2) * 32
                    ot, orow = h // 4, (h % 4) * 32
                    oa = [oap.tile([33, 512], f32, tag=f"oa{i}", name=f"oa{i}_{h}") for i in range(2)]
                    for jt in range(NT):
                        sc = [scp.tile([P, 512], f32, tag=f"sc{i}", name=f"sc{i}_{h}_{jt}") for i in range(2)]
                        for ih in range(2):
                            nc.tensor.matmul(
                                out=sc[ih][:],
                                lhsT=khT[ht][hr : hr + 32, jt * P : (jt + 1) * P],
                                rhs=qhT[ht][hr : hr + 32, ih * 512 : (ih + 1) * 512],
                                start=True,
                                stop=True,
                            )
                        # edge bias lands in window [0, n_cap) of i-half 0
                        nc.vector.tensor_tensor(
                            out=sc[0][:, :n_cap],
                            in0=sc[0][:, :n_cap],
                            in1=tb[jt][:, h::8],
                            op=ALU.add,
                        )
                        at_t = att.tile([P, N], f32, tag="attn")
                        for ih in range(2):
                            nc.vector.tensor_tensor(
                                out=sc[ih][:],
                                in0=sc[ih][:],
                                in1=mk[jt][:, ih * 512 : (ih + 1) * 512],
                                op=ALU.add,
                            )
                            nc.scalar.activation(
                                out=at_t[:, ih * 512 : (ih + 1) * 512],
                                in_=sc[ih][:],
                                func=AF.Exp,
                            )
                        for ih in range(2):
                            nc.tensor.matmul(
                                out=oa[ih][:],
                                lhsT=vha[jt][:, 33 * h : 33 * h + 33],
                                rhs=at_t[:, ih * 512 : (ih + 1) * 512],
                                start=(jt == 0),
                                stop=(jt == NT - 1),
                            )
                    # normalize: divide by rowsum (row 32), bcast via DRAM
                    rs = rsp.tile([1, N], f32, tag="rs")
                    for ih in range(2):
                        nc.scalar.activation(
                            out=rs[:, ih * 512 : (ih + 1) * 512],
                            in_=oa[ih][32:33, :],
                            func=AF.Copy,
                        )
                    rcp = rsp.tile([1, N], f32, tag="rcp")
                    nc.vector.reciprocal(out=rcp[:], in_=rs[:])
                    rs_d = dram_rs.tile([1, N], f32, tag="rsd")
                    nc.gpsimd.dma_start(out=rs_d[:], in_=rcp[:])
                    rb = rsp.tile([32, N], f32, tag="rb")
                    nc.gpsimd.dma_start(out=rb[:], in_=rs_d[0:1, :].to_broadcast((32, N)))
                    for ih in range(2):
                        nc.vector.tensor_tensor(
                            out=oT[ot][orow : orow + 32, ih * 512 : (ih + 1) * 512],
                            in0=oa[ih][0:32, :],
                            in1=rb[:, ih * 512 : (ih + 1) * 512],
                            op=ALU.mult,
                        )

            # ---------------- output projection ----------------
            with (
                tc.tile_pool(name="yp", bufs=4, space="PSUM") as yp,
                tc.tile_pool(name="ys", bufs=3) as ys,
            ):
                for nt in range(NT):
                    py = yp.tile([P, HID], f32, tag="py")
                    for ct in range(2):
                        nc.tensor.matmul(
                            out=py[:],
                            lhsT=oT[ct][:, nt * P : (nt + 1) * P],
                            rhs=wo_sb[ct][:],
                            start=(ct == 0),
                            stop=(ct == 1),
                        )
                    y_sb = ys.tile([P, HID], f32, tag="y")
                    nc.vector.tensor_tensor(
                        out=y_sb[:], in0=py[:], in1=bo_bc[:], op=ALU.add
                    )
                    nc.gpsimd.dma_start(out=out[nt * P : (nt + 1) * P, :], in_=y_sb[:])

    nc.compile()
    return nc


def _prep_edges(src, dst, starts, counts, n_cap, edge_attr, b):
    """Per-core edge chunking. Returns (slot_edge, slot_key, nc_u, nc_d)."""
    il = src - starts[b]
    key = (dst * n_cap + il).astype(np.int64)
    order = np.argsort(key, kind="stable")
    eid = order  # positions into this core's edge arrays
    keys = key[order]
    uq, inv, cnt = np.unique(keys, return_inverse=True, return_counts=True)
    is_single = (cnt == 1)[inv]

    s_eid, s_key = eid[is_single], keys[is_single]
    d_eid, d_key = eid[~is_single], keys[~is_single]

    nsing = len(s_eid)
    nc_u = (nsing + P - 1) // P if nsing else 0
    slots_e = []
    slots_k = []
    if nc_u:
        se = np.full(nc_u * P, -1, np.int64)
        sk = np.full(nc_u * P, -1, np.int64)
        se[:nsing] = s_eid
        sk[:nsing] = s_key
        slots_e.append(se)
        slots_k.append(sk)

    # pack duplicate groups into chunks without splitting a group
    dup_chunks_e = []
    dup_chunks_k = []
    if len(d_eid):
        # group boundaries (keys sorted)
        bounds = np.flatnonzero(np.diff(d_key)) + 1
        group_starts = np.concatenate([[0], bounds])
        group_ends = np.concatenate([bounds, [len(d_key)]])
        cur_e, cur_k, used = [], [], 0
        for gs, ge in zip(group_starts, group_ends):
            g = ge - gs
            if used + g > P:
                dup_chunks_e.append(np.concatenate(cur_e))
                dup_chunks_k.append(np.concatenate(cur_k))
                cur_e, cur_k, used = [], [], 0
            cur_e.append(d_eid[gs:ge])
            cur_k.append(d_key[gs:ge])
            used += g
        if used:
            dup_chunks_e.append(np.concatenate(cur_e))
            dup_chunks_k.append(np.concatenate(cur_k))
    nc_d = len(dup_chunks_e)
    for ce, ck in zip(dup_chunks_e, dup_chunks_k):
        se = np.full(P, -1, np.int64)
        sk = np.full(P, -1, np.int64)
        se[: len(ce)] = ce
        sk[: len(ck)] = ck
        slots_e.append(se)
        slots_k.append(sk)

    if slots_e:
        return np.concatenate(slots_e), np.concatenate(slots_k), nc_u, nc_d
    return np.empty(0, np.int64), np.empty(0, np.int64), 0, 0


def _prepare(inputs):
    q = np.ascontiguousarray(np.asarray(inputs["q"], np.float32))
    k = np.ascontiguousarray(np.asarray(inputs["k"], np.float32))
    v = np.ascontiguousarray(np.asarray(inputs["v"], np.float32))
    edge_attr = np.ascontiguousarray(np.asarray(inputs["edge_attr"], np.float32))
    edge_index = np.asarray(inputs["edge_index"]).astype(np.int64)
    batch = np.asarray(inputs["batch"]).astype(np.int64)
    attn_mask = np.asarray(inputs["attn_mask"]).astype(bool)
    Wq = np.asarray(inputs["Wq"], np.float32)
    Wk = np.asarray(inputs["Wk"], np.float32)
    Wv = np.asarray(inputs["Wv"], np.float32)
    We = np.asarray(inputs["We"], np.float32)
    Wo = np.asarray(inputs["Wo"], np.float32)
    bq = np.asarray(inputs["bq"], np.float32)
    bk = np.asarray(inputs["bk"], np.float32)
    bv = np.asarray(inputs["bv"], np.float32)
    be = np.asarray(inputs["be"], np.float32)
    bo = np.asarray(inputs["bo"], np.float32)

    counts = np.bincount(batch, minlength=B)
    starts = np.concatenate([[0], np.cumsum(counts)[:-1]]).astype(np.int64)
    n_cap = max(int(counts.max()), 8)

    src, dst = edge_index[0], edge_index[1]
    gid = batch[src]

    # per-core edge slots
    per_core = []
    for b in range(B):
        m = np.flatnonzero(gid == b)
        se, sk, ncu_b, ncd_b = _prep_edges(
            src[m], dst[m], starts, counts, n_cap, edge_attr, b
        )
        # se indexes into m
        per_core.append((m, se, sk, ncu_b, ncd_b))
    NCU = max(pc[3] for pc in per_core)
    NCD = max(pc[4] for pc in per_core)
    if NCU == 0:
        NCU = 1
    NC = NCU + NCD
    TRASH = np.int32(N * n_cap)

    key = (n_cap, NCU, NCD)
    if key in _PROGRAM_CACHE:
        nc = _PROGRAM_CACHE[key]
    else:
        nc = _build_program(n_cap, NCU, NCD)
        _PROGRAM_CACHE[key] = nc

    in_maps = []
    perms = []
    for b in range(B):
        m, se, sk, ncu_b, ncd_b = per_core[b]
        # assemble uniform (NC*P) slot arrays: unique chunks first, then dup
        slot_e = np.full(NC * P, -1, np.int64)
        slot_k = np.full(NC * P, -1, np.int64)
        nu = ncu_b * P
        slot_e[:nu] = se[:nu]
        slot_k[:nu] = sk[:nu]
        nd = (len(se) - nu)
        if nd:
            slot_e[NCU * P : NCU * P + nd] = se[nu:]
            slot_k[NCU * P : NCU * P + nd] = sk[nu:]
        valid = slot_e >= 0
        attrT_h = np.zeros((EF + 1, NC * P), np.float32)
        if valid.any():
            rows = m[slot_e[valid]]
            attrT_h[:EF, valid] = edge_attr[rows].T
            attrT_h[EF, valid] = 1.0
        idx_full = np.full(NC * P, TRASH, np.int32)
        idx_full[valid] = slot_k[valid].astype(np.int32)
        idxT_h = np.ascontiguousarray(idx_full.reshape(NC, P).T)

        s_b, n_b = int(starts[b]), int(counts[b])
        perm = np.concatenate(
            [np.arange(s_b, s_b + n_b), np.arange(0, s_b), np.arange(s_b + n_b, N)]
        )
        perms.append(perm)

        maskF_h = np.where(attn_mask[b].T[:, perm], np.float32(NEG), np.float32(0.0))

        in_maps.append(
            {
                "qT": np.ascontiguousarray(q[b].T[:, perm]),
                "kT": np.ascontiguousarray(k[b].T),
                "vT": np.ascontiguousarray(v[b].T),
                "maskF": np.ascontiguousarray(maskF_h),
                "WqT": np.ascontiguousarray(Wq.T),
                "WkT": np.ascontiguousarray(Wk.T),
                "WvT": np.ascontiguousarray(Wv.T),
                "WoT": np.ascontiguousarray(Wo.T),
                "bq": bq.reshape(HID, 1).copy(),
                "bk": bk.reshape(HID, 1).copy(),
                "bv": bv.reshape(1, HID).copy(),
                "bo": bo.reshape(1, HID).copy(),
                "WeTa": np.ascontiguousarray(
                    np.concatenate([We.T, be.reshape(1, H)], axis=0)
                ),
                "attrT": attrT_h,
                "idxT": idxT_h,
                "identI": np.eye(P, dtype=np.float32),
            }
        )

    return nc, in_maps, perms


def kernel(_trace=False, **inputs):
    nc, in_maps, perms = _prepare(inputs)
    from concourse.bass_utils import run_bass_kernel_spmd

    res = run_bass_kernel_spmd(
        nc, in_maps, core_ids=list(range(B)), trace=_trace
    )
    outs = []
    for b in range(B):
        y = res.results[b]["out"]
        inv = np.empty(N, np.int64)
        inv[perms[b]] = np.arange(N)
        outs.append(y[inv])
    final = np.stack(outs).astype(np.float32)
    if _trace:
        kernel._last_results = res
    return final
